# revision 12
# baseline (speedup 1.0000x reference)
"""Distributed causal self-attention kernel for 8 Trainium2 NeuronCores.

v2 (fp8 DoubleRow): 8 cores = 4 batches x 2 head-halves, as the fp16
baseline, but with the Tensor-engine work in fp8e4m3 DoubleRow wherever the
softmax attenuates quantization error:
  - QKV projections: x and the (x32 pre-scaled) weights in fp8, DoubleRow
    over contraction pairs [128, 2, .] -- half the instructions, descale by
    1/32 fused into the PSUM->SBUF casts (measured: a [256c,128,512] DR
    matmul costs the same 215.8ns as a [128c,128,512] fp16 one).
  - PV: P stored fp8 (exp writes fp8 directly), V stored fp8 in key-block
    PAIR layout [128, 2, 8*80]; one DR matmul covers 256 keys. Dead causal
    zones of the later block in a pair are filled with -30000 on the PE
    (exp -> 0) so pairs stay uniform.
  - fp16 corner path for queries 0-127 (too few keys for softmax averaging
    to attenuate fp8 error): fp16 v for token-block 0 via an fp16 projection,
    fp16 P, fp16 PV matmuls accumulated into the same PSUM accumulators.
  - S (contraction 64, row-packed head pairs), masks, and the out-projection
    stay fp16: fp8 gives no speed there (non-DR fp8 runs at fp16 speed) or
    costs too much accuracy (out-proj).
  - PE warmup burst during input DMA (HAM clock-gate), merged mask/fill
    matmuls ([128, 2, 128] strided PSUM out), single-recip normalization.
Measured: 270424 ns on 8 cores, absmax relative error 1.0e-2 (gate 2e-2).
"""

import numpy as np

B, T, D, H = 4, 2048, 1024, 16
HD = 64          # head dim
HPC = 8          # heads per core
DH = HPC * HD    # 512: head dims per core
NCORES = 8
SCALE = 1.0 / 32.0  # 1/sqrt(D)
# Schraudolph exp-approx constants: p ~= bitcast_f32(int32(s*SCH_A + SCH_B))
SCH_A = (1 << 23) * 1.4426950408889634 * SCALE
SCH_B = 127.0 * (1 << 23) - 366393.0

_cache = {}


def _build_nc():
    import concourse.bacc as bacc
    import concourse.mybir as mybir
    import concourse.tile as tile

    f8 = mybir.dt.float8e4
    f16 = mybir.dt.float16
    f32 = mybir.dt.float32
    Exp = mybir.ActivationFunctionType.Exp
    DR = mybir.MatmulPerfMode.DoubleRow

    nc = bacc.Bacc()
    Alu = mybir.AluOpType

    # fp8 operands, contraction k-pairs interleaved for DoubleRow
    x8 = nc.dram_tensor("x8", [512, 2, T], f8, kind="ExternalInput")
    wqk8 = nc.dram_tensor("wqk8", [512, 2, 2 * DH], f8, kind="ExternalInput")
    wv8 = nc.dram_tensor("wv8", [512, 2, DH], f8, kind="ExternalInput")
    # fp16 corner operands: x^T tokens 0-127 and full wv
    xc16 = nc.dram_tensor("xc16", [D, 128], f16, kind="ExternalInput")
    wv = nc.dram_tensor("wv", [D, DH], f16, kind="ExternalInput")
    wo = nc.dram_tensor("wo", [DH, D], f16, kind="ExternalInput")
    bqk = nc.dram_tensor("bqk", [128, 8], f32, kind="ExternalInput")
    bv = nc.dram_tensor("bv", [128, DH], f32, kind="ExternalInput")
    maskb = nc.dram_tensor("maskb", [128, 128], f16, kind="ExternalInput")
    ident = nc.dram_tensor("ident", [128, 128], f16, kind="ExternalInput")
    y = nc.dram_tensor("y", [T, D], f32, kind="ExternalOutput")

    x8_t = x8.rearrange("(k p) g t -> k p g t", p=128)     # 4 x [128, 2, 2048]
    wqk8_t = wqk8.rearrange("(k p) g n -> k p g n", p=128)  # 4 x [128, 2, 1024]
    wv8_t = wv8.rearrange("(k p) g n -> k p g n", p=128)    # 4 x [128, 2, 512]
    xc16_t = xc16.rearrange("(k p) t -> k p t", p=128)      # 8 x [128, 128]
    wv_t = wv.rearrange("(k p) n -> k p n", p=128)     # 8 x [128, 512]
    wo_t = wo.rearrange("(k p) n -> k p n", p=128)     # 4 x [128, 1024]

    with tile.TileContext(nc) as tc:
        with (
            tc.tile_pool(name="consts", bufs=1) as cp,
            tc.tile_pool(name="ptp", bufs=6) as ptp,
            tc.tile_pool(name="ptp16", bufs=2) as ptp16,
            tc.tile_pool(name="recp", bufs=4) as recp,
            tc.tile_pool(name="ysb", bufs=3) as ysb,
            tc.tile_pool(name="psum", bufs=2, space="PSUM") as psp,
        ):
            xt2_sb = [cp.tile([128, 2, T], f8, name=f"xt2_{k}", tag=f"xt2_{k}") for k in range(4)]
            wqk8_sb = [cp.tile([128, 2, 2 * DH], f8, name=f"wqk8_{k}", tag=f"wqk8_{k}") for k in range(4)]
            wv8_sb = [cp.tile([128, 2, DH], f8, name=f"wv8_{k}", tag=f"wv8_{k}") for k in range(4)]
            xc16_sb = cp.tile([128, 8, 128], f16, name="xc16_sb", tag="xc16_sb")
            wv_sb = [cp.tile([128, DH], f16, name=f"wv{k}", tag=f"wv{k}") for k in range(8)]
            wo_sb = [cp.tile([128, D], f16, name=f"wo{k}", tag=f"wo{k}") for k in range(4)]
            bqk_sb = cp.tile([128, 8], f32, name="bqk_sb", tag="bqk_sb")
            bv_sb = cp.tile([128, DH], f32, name="bv_sb", tag="bv_sb")
            maskb_sb = cp.tile([128, 256], f16, name="maskb_sb", tag="maskb_sb")
            ident_sb = cp.tile([128, 128], f16, name="ident_sb", tag="ident_sb")
            neg30k_sb = cp.tile([128, 256], f16, name="neg30k_sb", tag="neg30k_sb")
            warm_sb = cp.tile([128, 640], f16, name="warm_sb", tag="warm_sb")
            # q^T on tiles 0-3 (head pairs), k^T on tiles 4-7; fp8, head-dim
            # split [32p, 2g]; pair 2P primary at partitions 0-63 (a 0-31,
            # b 32-63), pair 2P+1 primary at 64-127; the other half holds a
            # DMA-duplicated copy so S blocks can alternate PE row positions
            qk8_sb = [cp.tile([128, 2, T], f8, name=f"qk8_{r}", tag=f"qk8_{r}")
                      for r in range(8)]
            # v fp8, key-block pairs: [128, 2, 8*80] (64 dims + ones + 15 pad)
            v2_sb = [cp.tile([128, 2, HPC * 80], f8, name=f"v{t}", tag=f"v{t}")
                     for t in range(8)]
            # fp16 v for token block 0 (corner path), 65 cols per head
            v16_sb = cp.tile([128, HPC * 65], f16, name="v16_sb", tag="v16_sb")
            oT_sb = [cp.tile([128, T], f16, name=f"oT{i}", tag=f"oT{i}") for i in range(4)]

            for k in range(4):
                nc.sync.dma_start(xt2_sb[k][:], x8_t[k])
                nc.sync.dma_start(wqk8_sb[k][:], wqk8_t[k])
                nc.sync.dma_start(wv8_sb[k][:], wv8_t[k])
            for k in range(8):
                nc.sync.dma_start(xc16_sb[:, k, :], xc16_t[k])
                nc.sync.dma_start(wv_sb[k][:], wv_t[k])
            nc.sync.dma_start(bqk_sb[:], bqk[:])
            nc.sync.dma_start(bv_sb[:], bv[:])
            nc.sync.dma_start(maskb_sb[:, 0:128], maskb[:])
            nc.sync.dma_start(maskb_sb[:, 128:256], maskb[:])
            nc.sync.dma_start(ident_sb[:], ident[:])
            for k in range(4):
                nc.sync.dma_start(wo_sb[k][:], wo_t[k])

            # PE warmup: dummy matmuls on a scratch tile while input DMAs
            # land -- gets the HAM clock gate to 8/8 before real work
            nc.gpsimd.memset(warm_sb[:], 0.0)
            warmp = psp.tile([128, 512], f32, name="warmp", tag="mm", bufs=3)
            for wi in range(36):
                nc.tensor.matmul(
                    warmp[:], warm_sb[:, 0:128], warm_sb[:, 128:640],
                    start=True, stop=True)

            nc.gpsimd.memset(neg30k_sb[:], -30000.0)
            # ones columns (col 64 of each 80-col head group; both key groups)
            for t in range(8):
                vv = v2_sb[t][:].rearrange("p g (h c) -> p g h c", c=80)
                nc.gpsimd.memset(vv[:, :, :, 64:65], 1.0)
            v16v = v16_sb[:].rearrange("p (h c) -> p h c", c=65)
            nc.gpsimd.memset(v16v[:, :, 64:65], 1.0)

            # ---- emit helpers ----
            def emit_qk_pp(qk, P, tb):
                # q^T or k^T for head-pairs 2P,2P+1, token block tb (512).
                # fp8 DoubleRow; psum [128, 2(g), 512]: partitions 0-63 =
                # pair 2P dims (a|b x 32), 64-127 = pair 2P+1. Weights
                # pre-scaled x32 and column-permuted on the host.
                pt = psp.tile([128, 2, 512], f32, name=f"qkp{qk}_{P}_{tb}",
                              tag="mm", bufs=3)
                for g in range(2):
                    b8 = qk * 4 + P * 2 + g
                    for k in range(4):
                        nc.tensor.matmul(
                            pt[:, g, :],
                            wqk8_sb[k][:, :, b8 * 128:(b8 + 1) * 128],
                            xt2_sb[k][:, :, tb * 512:(tb + 1) * 512],
                            start=(k == 0), stop=(k == 3), perf_mode=DR,
                        )
                ts_ = slice(tb * 512, (tb + 1) * 512)
                tA = qk8_sb[qk * 4 + 2 * P]
                tB = qk8_sb[qk * 4 + 2 * P + 1]
                for g in range(2):
                    b8 = qk * 4 + P * 2 + g
                    nc.vector.tensor_scalar(
                        tA[0:64, g, ts_], pt[0:64, g, :],
                        1.0 / 32.0, bqk_sb[0:64, b8:b8 + 1], Alu.mult, Alu.add)
                    nc.vector.tensor_scalar(
                        tB[64:128, g, ts_], pt[64:128, g, :],
                        1.0 / 32.0, bqk_sb[64:128, b8:b8 + 1], Alu.mult, Alu.add)
                # duplicate into the other partition half (row-position copies)
                nc.sync.dma_start(tA[64:128, :, ts_], tA[0:64, :, ts_])
                nc.sync.dma_start(tB[0:64, :, ts_], tB[64:128, :, ts_])

            bv_b = bv_sb[:].rearrange("p (h c) -> p h c", c=64)

            def emit_v(tt):
                # v for token tiles 2tt, 2tt+1 = xT.T @ wv + bias -> fp8 v2
                # (fp8 DoubleRow, wv pre-scaled x32 -> descale on copy)
                pv = psp.tile([128, 1024], f32, name=f"vp{tt}", tag="mm", bufs=3)
                for k in range(4):
                    for h in range(2):
                        t = 2 * tt + h
                        nc.tensor.matmul(
                            pv[:, h * 512:(h + 1) * 512],
                            xt2_sb[k][:, :, t * 128:(t + 1) * 128],
                            wv8_sb[k][:],
                            start=(k == 0), stop=(k == 3), perf_mode=DR,
                        )
                for h in range(2):
                    vdst = v2_sb[tt][:, h, :].rearrange(
                        "p (h2 c) -> p h2 c", c=80)[:, :, 0:64]
                    vsrc = pv[:, h * 512:(h + 1) * 512].rearrange(
                        "p (h2 c) -> p h2 c", c=64)
                    nc.vector.scalar_tensor_tensor(
                        vdst, vsrc, 1.0 / 32.0, bv_b, Alu.mult, Alu.add)
                if tt == 0:
                    # corner: accurate fp16 v for token block 0
                    pc = psp.tile([128, 512], f32, name="vc16p", tag="mm", bufs=3)
                    for k in range(8):
                        nc.tensor.matmul(
                            pc[:], xc16_sb[:, k, :], wv_sb[k][:],
                            start=(k == 0), stop=(k == 7),
                        )
                    v16dst = v16_sb[:].rearrange("p (h2 c) -> p h2 c", c=65)[:, :, 0:64]
                    vsrc0 = pc[:].rearrange("p (h2 c) -> p h2 c", c=64)
                    nc.vector.tensor_add(v16dst, vsrc0, bv_b)

            accum = {}
            sums = {}
            p8s = {}
            p16s = {}

            def emit_attn_S(i, g, j, opair):
                # S^T psum tile [128, 2*512] (head a | head b), keys of block j
                # fp8 DoubleRow [32p, 2g] contraction; row position alternates
                # with j parity (pair primary vs copy partition half) so four
                # S streams overlap on the PE array
                o = max(0, 128 * j - 512 * g)
                w = 512 - o
                q0 = 512 * g + o
                part = j >= 4 * g   # partial block: needs causal mask
                qt = qk8_sb[i]
                kt = qk8_sb[4 + i]
                base = 64 * ((i % 2) ^ (j % 2))
                pa, pb = base, base + 32
                sab = psp.tile([128, 1024], f32, name=f"sab{i}_{g}_{j}",
                               tag="mm", bufs=3)
                nc.tensor.matmul(
                    sab[:, o:512], kt[pa:pa + 32, :, j * 128:(j + 1) * 128],
                    qt[pa:pa + 32, :, q0:q0 + w], start=True, stop=not part,
                    perf_mode=DR, tile_position=(pa, 0))
                nc.tensor.matmul(
                    sab[:, 512 + o:1024], kt[pb:pb + 32, :, j * 128:(j + 1) * 128],
                    qt[pb:pb + 32, :, q0:q0 + w], start=True, stop=not part,
                    perf_mode=DR, tile_position=(pb, 0))
                sab2 = sab[:].rearrange("p (h q) -> p h q", h=2)
                if part:
                    # add -30000 above the diagonal via identity matmul
                    # (both heads in one matmul: strided [128, 2, 128] out)
                    nc.tensor.matmul(
                        sab2[:, :, o:o + 128], ident_sb[:], maskb_sb[:],
                        start=False, stop=True, skip_group_check=True)
                if o > opair and not (g == 0 and j // 2 == 0):
                    # fill [opair, o) with -30000 so exp -> 0 (dead causal zone
                    # of the later block in a PV pair). start=False: the bank
                    # was already started by the S matmul (start=True clears
                    # the WHOLE 2KB bank); this accumulates onto pending-zero.
                    fw = o - opair
                    mv = neg30k_sb[:].rearrange("p (h q) -> p h q", h=2)[:, :, 0:fw]
                    nc.tensor.matmul(
                        sab2[:, :, opair:opair + fw], ident_sb[:],
                        mv, start=False, stop=True,
                        skip_group_check=True)
                jj = j // 2
                js = j % 2
                if js == 0:
                    p8 = ptp.tile([128, 2, 2, 512], f8, name=f"p8_{i}_{g}_{jj}",
                                  tag="p8")
                    p8s[(i, g, jj)] = p8
                p8 = p8s[(i, g, jj)]
                corner = (g == 0 and j == 0)
                e0 = 128 if (g == 0 and jj == 0) else opair
                # exp into fp8 P slot (both heads at once)
                sv = sab[:].rearrange("p (h q) -> p h q", h=2)[:, :, e0:512]
                pv_ = p8[:, js, :, e0:512]
                nc.scalar.activation(pv_, sv, Exp, scale=SCALE)
                if corner:
                    p16 = ptp16.tile([128, 2, 128], f16, name=f"p16_{i}",
                                     tag="p16")
                    sv0 = sab[:].rearrange("p (h q) -> p h q", h=2)[:, :, 0:128]
                    nc.scalar.activation(p16[:], sv0, Exp, scale=SCALE)
                    p16s[i] = p16

            def emit_attn_PV(i, g, jj, opair, first, last):
                # fp8 DoubleRow: two key blocks (2jj, 2jj+1) per instruction
                if first:
                    oa = psp.tile([65, 512], f32, name=f"oa{i}_{g}", tag="pva", bufs=1)
                    ob = psp.tile([65, 512], f32, name=f"ob{i}_{g}", tag="pvb", bufs=1)
                    accum[(i, g)] = (oa, ob)
                oa, ob = accum[(i, g)]
                p8 = p8s.pop((i, g, jj))
                va = v2_sb[jj][:, :, (2 * i) * 80:(2 * i) * 80 + 65]
                vb = v2_sb[jj][:, :, (2 * i + 1) * 80:(2 * i + 1) * 80 + 65]
                corner = (g == 0 and jj == 0)
                e0 = 128 if corner else opair
                nc.tensor.matmul(
                    oa[:, e0:512], va, p8[:, :, 0, e0:512],
                    start=first, stop=last, perf_mode=DR,
                    skip_group_check=True)
                nc.tensor.matmul(
                    ob[:, e0:512], vb, p8[:, :, 1, e0:512],
                    start=first, stop=last, perf_mode=DR,
                    skip_group_check=True)
                if corner:
                    # start=False: bank was started (and fully cleared) by the
                    # DR pair instruction above; accumulate onto pending-zero.
                    p16 = p16s.pop(i)
                    nc.tensor.matmul(
                        oa[:, 0:128], v16_sb[:, (2 * i) * 65:(2 * i) * 65 + 65],
                        p16[:, 0, :], start=False, stop=True,
                        skip_group_check=True)
                    nc.tensor.matmul(
                        ob[:, 0:128], v16_sb[:, (2 * i + 1) * 65:(2 * i + 1) * 65 + 65],
                        p16[:, 1, :], start=False, stop=True,
                        skip_group_check=True)

            def emit_attn_norm(i, g):
                # fast part: copy unnormalized O + denominators out of PSUM,
                # releasing the accumulators quickly
                oa, ob = accum.pop((i, g))
                sab_in = recp.tile([1, 1024], f32, name=f"sin{i}_{g}", tag="sin")
                g0 = 512 * g
                nc.vector.tensor_copy(oT_sb[i][0:64, g0:g0 + 512], oa[0:64, :])
                nc.vector.tensor_copy(oT_sb[i][64:128, g0:g0 + 512], ob[0:64, :])
                nc.vector.tensor_copy(sab_in[:, 0:512], oa[64:65, :])
                nc.vector.tensor_copy(sab_in[:, 512:1024], ob[64:65, :])
                sums[(i, g)] = sab_in

            def emit_attn_scale(i, g):
                # lazy part: 1/denom, broadcast, scale oT in place
                sab_in = sums.pop((i, g))
                rab = recp.tile([1, 1024], f32, name=f"rab{i}_{g}", tag="rab")
                rr = recp.tile([128, 512], f32, name=f"rr{i}_{g}", tag="rr")
                rbb = recp.tile([64, 512], f32, name=f"rbb{i}_{g}", tag="rbb")
                nc.vector.reciprocal_approx_fast(rab[:], sab_in[:])
                nc.gpsimd.partition_broadcast(rr[0:64, :], rab[:, 0:512], channels=64)
                nc.gpsimd.partition_broadcast(rbb[:], rab[:, 512:1024])
                nc.sync.dma_start(rr[64:128, :], rbb[:])
                g0 = 512 * g
                nc.vector.tensor_mul(
                    oT_sb[i][:, g0:g0 + 512],
                    oT_sb[i][:, g0:g0 + 512], rr[:])

            def emit_attn_gg(i, gg, vmap=None):
                # software-pipelined: each pair's PV is emitted after the NEXT
                # pair's S matmuls, so the exp latency hides behind PE work
                for g in (gg, gg + 1):
                    nj = 4 * g + 4
                    npair = nj // 2
                    pending = None
                    for jj in range(npair):
                        j0, j1 = 2 * jj, 2 * jj + 1
                        opair = max(0, 128 * j0 - 512 * g)
                        for j in (j0, j1):
                            if vmap and (g, j) in vmap:
                                emit_v(vmap[(g, j)])
                            emit_attn_S(i, g, j, opair)
                        if pending is not None:
                            emit_attn_PV(*pending)
                        pending = (i, g, jj, opair, jj == 0, jj == npair - 1)
                    emit_attn_PV(*pending)
                    emit_attn_norm(i, g)
                    emit_attn_scale(i, g)

            def emit_out(t):
                pt_ = psp.tile([128, 1024], f32, name=f"yp{t}", tag="mm", bufs=3)
                for k in range(4):
                    for h2 in range(2):
                        nc.tensor.matmul(
                            pt_[:, h2 * 512:(h2 + 1) * 512],
                            oT_sb[k][:, t * 128:(t + 1) * 128],
                            wo_sb[k][:, h2 * 512:(h2 + 1) * 512],
                            start=(k == 0), stop=(k == 3),
                        )
                yt = ysb.tile([128, D], f32, name=f"yt{t}", tag="yt")
                nc.vector.tensor_copy(yt[:], pt_[:])
                nc.sync.dma_start(y[t * 128:(t + 1) * 128, :], yt[:])

            # ---- interleaved emission ----
            emit_qk_pp(0, 0, 0); emit_qk_pp(0, 0, 1)
            emit_qk_pp(1, 0, 0); emit_qk_pp(1, 0, 1)
            emit_v(0); emit_v(1)
            emit_attn_gg(0, 0, vmap={(1, 4): 2, (1, 6): 3})
            emit_qk_pp(0, 0, 2); emit_qk_pp(0, 0, 3)
            emit_qk_pp(1, 0, 2); emit_qk_pp(1, 0, 3)
            emit_attn_gg(0, 2, vmap={(2, 8): 4, (2, 10): 5, (3, 12): 6, (3, 14): 7})
            emit_qk_pp(0, 1, 0); emit_qk_pp(0, 1, 1)
            emit_attn_gg(1, 0)
            emit_qk_pp(1, 1, 0); emit_qk_pp(1, 1, 1)
            emit_attn_gg(1, 2)
            emit_qk_pp(0, 1, 2); emit_qk_pp(0, 1, 3)
            emit_attn_gg(2, 0)
            emit_qk_pp(1, 1, 2); emit_qk_pp(1, 1, 3)
            emit_attn_gg(2, 2)
            emit_attn_gg(3, 0)
            for t in range(8):
                emit_out(t)
            emit_attn_gg(3, 2)
            for t in range(8, 16):
                emit_out(t)

    nc.compile()
    return nc


def _interleave_k(a):
    # [1024, N...] -> [512, 2, N...]: contraction k-pairs for DoubleRow
    # row (128*kk + p, g) <- original row 256*kk + 128*g + p
    n = a.shape[1:]
    return np.ascontiguousarray(
        a.reshape(4, 2, 128, *n).transpose(0, 2, 1, 3).reshape(512, 2, *n))


def _prep_inputs(x, w_qkv, b_qkv, w_out):
    import ml_dtypes
    f8 = ml_dtypes.float8_e4m3
    pidx = np.arange(128)
    maskb_np = np.where(pidx[None, :] >= pidx[:, None], 0.0, -30000.0).astype(np.float16)
    ident_np = np.eye(128, dtype=np.float16)
    in_maps = []
    for c in range(NCORES):
        b, hh = divmod(c, 2)
        h0 = hh * HPC * HD  # 0 or 512: offset into each of q/k/v col sections
        wq = w_qkv[:, h0:h0 + DH]
        wk = w_qkv[:, D + h0:D + h0 + DH]
        wv_ = w_qkv[:, 2 * D + h0:2 * D + h0 + DH]
        bq = b_qkv[h0:h0 + DH]
        bk = b_qkv[D + h0:D + h0 + DH]
        bv_ = b_qkv[2 * D + h0:2 * D + h0 + DH]
        xb = np.ascontiguousarray(x[b].T)  # [D, T]
        wqkc = np.concatenate([wq, wk], axis=1)
        # permute wqk columns into (qk, P, g) blocks of 128: within a block,
        # col c -> head 4P + c//32, dim 32g + c%32 (psum partition layout)
        newcols = np.empty(1024, dtype=np.int64)
        for j8 in range(8):
            qk_, P_, g_ = j8 // 4, (j8 % 4) // 2, j8 % 2
            c = np.arange(128)
            newcols[j8 * 128:(j8 + 1) * 128] = (
                qk_ * 512 + (4 * P_ + c // 32) * 64 + 32 * g_ + c % 32)
        bqk_vec = np.concatenate([bq, bk])
        bqk8 = np.empty((128, 8), dtype=np.float32)
        for b8 in range(8):
            qk_, P_, g_ = b8 // 4, (b8 % 4) // 2, b8 % 2
            p = np.arange(128)
            head = 4 * P_ + 2 * (p // 64) + (p % 64) // 32
            d = 32 * g_ + p % 32
            bqk8[:, b8] = bqk_vec[qk_ * 512 + head * 64 + d]
        in_maps.append({
            "x8": _interleave_k(xb).astype(f8),
            "wqk8": _interleave_k(wqkc[:, newcols] * 32.0).astype(f8),
            "wv8": _interleave_k(wv_ * 32.0).astype(f8),
            "xc16": np.ascontiguousarray(xb[:, 0:128]).astype(np.float16),
            "wv": np.ascontiguousarray(wv_).astype(np.float16),
            "wo": np.ascontiguousarray(w_out[h0:h0 + DH, :]).astype(np.float16),
            "bqk": bqk8,
            "bv": np.broadcast_to(bv_.astype(np.float32), (128, DH)).copy(),
            "maskb": maskb_np, "ident": ident_np,
        })
    return in_maps


def kernel(x, w_qkv, b_qkv, w_out, b_out, _trace=False, _trace_cores=None):
    from concourse.bass_utils import run_bass_kernel_spmd

    x = np.asarray(x, dtype=np.float32)
    w_qkv = np.asarray(w_qkv, dtype=np.float32)
    b_qkv = np.asarray(b_qkv, dtype=np.float32)
    w_out = np.asarray(w_out, dtype=np.float32)
    b_out = np.asarray(b_out, dtype=np.float32)

    if "nc" not in _cache:
        _cache["nc"] = _build_nc()
    nc = _cache["nc"]

    in_maps = _prep_inputs(x, w_qkv, b_qkv, w_out)
    res = run_bass_kernel_spmd(
        nc, in_maps, core_ids=list(range(NCORES)), trace=_trace,
        trace_cores=_trace_cores)
    _cache["last_result"] = res

    out = np.empty((B, T, D), dtype=np.float32)
    for b in range(B):
        out[b] = res.results[2 * b]["y"] + res.results[2 * b + 1]["y"]
    out += b_out[None, None, :].astype(np.float32)
    return out


# revision 13
# speedup vs baseline: 1.1935x; 1.1935x over previous
"""Distributed causal self-attention kernel for 8 Trainium2 NeuronCores.

v2 (fp8 DoubleRow): 8 cores = 4 batches x 2 head-halves, as the fp16
baseline, but with the Tensor-engine work in fp8e4m3 DoubleRow wherever the
softmax attenuates quantization error:
  - QKV projections: x and the (x32 pre-scaled) weights in fp8, DoubleRow
    over contraction pairs [128, 2, .] -- half the instructions, descale by
    1/32 fused into the PSUM->SBUF casts (measured: a [256c,128,512] DR
    matmul costs the same 215.8ns as a [128c,128,512] fp16 one).
  - PV: P stored fp8 (exp writes fp8 directly), V stored fp8 in key-block
    PAIR layout [128, 2, 8*80]; one DR matmul covers 256 keys. Dead causal
    zones of the later block in a pair are filled with -30000 on the PE
    (exp -> 0) so pairs stay uniform.
  - fp16 corner path for queries 0-127 (too few keys for softmax averaging
    to attenuate fp8 error): fp16 v for token-block 0 via an fp16 projection,
    fp16 P, fp16 PV matmuls accumulated into the same PSUM accumulators.
  - S (contraction 64, row-packed head pairs), masks, and the out-projection
    stay fp16: fp8 gives no speed there (non-DR fp8 runs at fp16 speed) or
    costs too much accuracy (out-proj).
  - PE warmup burst during input DMA (HAM clock-gate), merged mask/fill
    matmuls ([128, 2, 128] strided PSUM out), single-recip normalization.
Measured: 270424 ns on 8 cores, absmax relative error 1.0e-2 (gate 2e-2).
"""

import numpy as np

B, T, D, H = 4, 2048, 1024, 16
HD = 64          # head dim
HPC = 8          # heads per core
DH = HPC * HD    # 512: head dims per core
NCORES = 8
SCALE = 1.0 / 32.0  # 1/sqrt(D)
# Schraudolph exp-approx constants: p ~= bitcast_f32(int32(s*SCH_A + SCH_B))
SCH_A = (1 << 23) * 1.4426950408889634 * SCALE
SCH_B = 127.0 * (1 << 23) - 366393.0

_cache = {}


def _build_nc():
    import concourse.bacc as bacc
    import concourse.mybir as mybir
    import concourse.tile as tile

    f8 = mybir.dt.float8e4
    f16 = mybir.dt.float16
    f32 = mybir.dt.float32
    Exp = mybir.ActivationFunctionType.Exp
    DR = mybir.MatmulPerfMode.DoubleRow

    nc = bacc.Bacc()
    Alu = mybir.AluOpType

    # fp8 operands, contraction k-pairs interleaved for DoubleRow
    x8 = nc.dram_tensor("x8", [512, 2, T], f8, kind="ExternalInput")
    wqk8 = nc.dram_tensor("wqk8", [512, 2, 2 * DH], f8, kind="ExternalInput")
    wv8 = nc.dram_tensor("wv8", [512, 2, DH], f8, kind="ExternalInput")
    # fp16 corner operands: x^T tokens 0-127 and full wv
    xc16 = nc.dram_tensor("xc16", [D, 128], f16, kind="ExternalInput")
    wv = nc.dram_tensor("wv", [D, DH], f16, kind="ExternalInput")
    wo = nc.dram_tensor("wo", [DH, D], f16, kind="ExternalInput")
    bqk = nc.dram_tensor("bqk", [128, 8], f32, kind="ExternalInput")
    bv = nc.dram_tensor("bv", [128, DH], f32, kind="ExternalInput")
    maskb = nc.dram_tensor("maskb", [128, 128], f16, kind="ExternalInput")
    ident = nc.dram_tensor("ident", [128, 128], f16, kind="ExternalInput")
    y = nc.dram_tensor("y", [T, D], f32, kind="ExternalOutput")

    x8_t = x8.rearrange("(k p) g t -> k p g t", p=128)     # 4 x [128, 2, 2048]
    wqk8_t = wqk8.rearrange("(k p) g n -> k p g n", p=128)  # 4 x [128, 2, 1024]
    wv8_t = wv8.rearrange("(k p) g n -> k p g n", p=128)    # 4 x [128, 2, 512]
    xc16_t = xc16.rearrange("(k p) t -> k p t", p=128)      # 8 x [128, 128]
    wv_t = wv.rearrange("(k p) n -> k p n", p=128)     # 8 x [128, 512]
    wo_t = wo.rearrange("(k p) n -> k p n", p=128)     # 4 x [128, 1024]

    with tile.TileContext(nc) as tc:
        with (
            tc.tile_pool(name="consts", bufs=1) as cp,
            tc.tile_pool(name="ptp", bufs=4) as ptp,
            tc.tile_pool(name="ptp16", bufs=2) as ptp16,
            tc.tile_pool(name="recp", bufs=4) as recp,
            tc.tile_pool(name="ysb", bufs=3) as ysb,
            tc.tile_pool(name="psum", bufs=2, space="PSUM") as psp,
        ):
            xt2_sb = [cp.tile([128, 2, T], f8, name=f"xt2_{k}", tag=f"xt2_{k}") for k in range(4)]
            wqk8_sb = [cp.tile([128, 2, 2 * DH], f8, name=f"wqk8_{k}", tag=f"wqk8_{k}") for k in range(4)]
            wv8_sb = [cp.tile([128, 2, DH], f8, name=f"wv8_{k}", tag=f"wv8_{k}") for k in range(4)]
            xc16_sb = cp.tile([128, 8, 128], f16, name="xc16_sb", tag="xc16_sb")
            wv_sb = [cp.tile([128, DH], f16, name=f"wv{k}", tag=f"wv{k}") for k in range(8)]
            wo_sb = [cp.tile([128, D], f16, name=f"wo{k}", tag=f"wo{k}") for k in range(4)]
            bqk_sb = cp.tile([128, 8], f32, name="bqk_sb", tag="bqk_sb")
            bv_sb = cp.tile([128, DH], f32, name="bv_sb", tag="bv_sb")
            maskb_sb = cp.tile([128, 256], f16, name="maskb_sb", tag="maskb_sb")
            ident_sb = cp.tile([128, 128], f16, name="ident_sb", tag="ident_sb")
            neg30k_sb = cp.tile([128, 256], f16, name="neg30k_sb", tag="neg30k_sb")
            warm_sb = cp.tile([128, 640], f16, name="warm_sb", tag="warm_sb")
            # q^T on tiles 0-3 (head pairs), k^T on tiles 4-7
            qk_sb = [cp.tile([128, T], f16, name=f"qk{r}", tag=f"qk{r}") for r in range(8)]
            # v fp8, key-block pairs: [128, 2, 8*80] (64 dims + ones + 15 pad)
            v2_sb = [cp.tile([128, 2, HPC * 80], f8, name=f"v{t}", tag=f"v{t}")
                     for t in range(8)]
            # fp16 v for token block 0 (corner path), 65 cols per head
            v16_sb = cp.tile([128, HPC * 65], f16, name="v16_sb", tag="v16_sb")
            oT_sb = [cp.tile([128, T], f16, name=f"oT{i}", tag=f"oT{i}") for i in range(4)]

            for k in range(4):
                nc.sync.dma_start(xt2_sb[k][:], x8_t[k])
                nc.sync.dma_start(wqk8_sb[k][:], wqk8_t[k])
                nc.sync.dma_start(wv8_sb[k][:], wv8_t[k])
            for k in range(8):
                nc.sync.dma_start(xc16_sb[:, k, :], xc16_t[k])
                nc.sync.dma_start(wv_sb[k][:], wv_t[k])
            nc.sync.dma_start(bqk_sb[:], bqk[:])
            nc.sync.dma_start(bv_sb[:], bv[:])
            nc.sync.dma_start(maskb_sb[:, 0:128], maskb[:])
            nc.sync.dma_start(maskb_sb[:, 128:256], maskb[:])
            nc.sync.dma_start(ident_sb[:], ident[:])
            for k in range(4):
                nc.sync.dma_start(wo_sb[k][:], wo_t[k])

            # PE warmup: dummy matmuls on a scratch tile while input DMAs
            # land -- gets the HAM clock gate to 8/8 before real work
            nc.gpsimd.memset(warm_sb[:], 0.0)
            warmp = psp.tile([128, 512], f32, name="warmp", tag="mm", bufs=3)
            for wi in range(36):
                nc.tensor.matmul(
                    warmp[:], warm_sb[:, 0:128], warm_sb[:, 128:640],
                    start=True, stop=True)

            nc.gpsimd.memset(neg30k_sb[:], -30000.0)
            # ones columns (col 64 of each 80-col head group; both key groups)
            for t in range(8):
                vv = v2_sb[t][:].rearrange("p g (h c) -> p g h c", c=80)
                nc.gpsimd.memset(vv[:, :, :, 64:65], 1.0)
            v16v = v16_sb[:].rearrange("p (h c) -> p h c", c=65)
            nc.gpsimd.memset(v16v[:, :, 64:65], 1.0)

            # ---- emit helpers ----
            def emit_qk_row(r, c2):
                # q^T / k^T row tile r, token half c2 = wqk.T @ xT + bias
                # (fp8 DoubleRow, weights pre-scaled x32 -> descale on copy)
                pt = psp.tile([128, 1024], f32, name=f"qkp{r}_{c2}", tag="mm", bufs=3)
                for k in range(4):
                    for h in range(2):
                        c = c2 * 2 + h
                        nc.tensor.matmul(
                            pt[:, h * 512:h * 512 + 512],
                            wqk8_sb[k][:, :, r * 128:(r + 1) * 128],
                            xt2_sb[k][:, :, c * 512:(c + 1) * 512],
                            start=(k == 0), stop=(k == 3), perf_mode=DR,
                        )
                nc.vector.tensor_scalar(
                    qk_sb[r][:, c2 * 1024:(c2 + 1) * 1024], pt[:],
                    1.0 / 32.0, bqk_sb[:, r:r + 1], Alu.mult, Alu.add,
                )

            bv_b = bv_sb[:].rearrange("p (h c) -> p h c", c=64)

            def emit_v(tt):
                # v for token tiles 2tt, 2tt+1 = xT.T @ wv + bias -> fp8 v2
                # (fp8 DoubleRow, wv pre-scaled x32 -> descale on copy)
                pv = psp.tile([128, 1024], f32, name=f"vp{tt}", tag="mm", bufs=3)
                for k in range(4):
                    for h in range(2):
                        t = 2 * tt + h
                        nc.tensor.matmul(
                            pv[:, h * 512:(h + 1) * 512],
                            xt2_sb[k][:, :, t * 128:(t + 1) * 128],
                            wv8_sb[k][:],
                            start=(k == 0), stop=(k == 3), perf_mode=DR,
                        )
                for h in range(2):
                    vdst = v2_sb[tt][:, h, :].rearrange(
                        "p (h2 c) -> p h2 c", c=80)[:, :, 0:64]
                    vsrc = pv[:, h * 512:(h + 1) * 512].rearrange(
                        "p (h2 c) -> p h2 c", c=64)
                    nc.vector.scalar_tensor_tensor(
                        vdst, vsrc, 1.0 / 32.0, bv_b, Alu.mult, Alu.add)
                if tt == 0:
                    # corner: accurate fp16 v for token block 0
                    pc = psp.tile([128, 512], f32, name="vc16p", tag="mm", bufs=3)
                    for k in range(8):
                        nc.tensor.matmul(
                            pc[:], xc16_sb[:, k, :], wv_sb[k][:],
                            start=(k == 0), stop=(k == 7),
                        )
                    v16dst = v16_sb[:].rearrange("p (h2 c) -> p h2 c", c=65)[:, :, 0:64]
                    vsrc0 = pc[:].rearrange("p (h2 c) -> p h2 c", c=64)
                    nc.vector.tensor_add(v16dst, vsrc0, bv_b)

            accum = {}
            sums = {}
            p8s = {}
            p16s = {}

            def emit_attn_S(i, g, j, opair):
                # S^T psum tile [128, 2*512] (head a | head b), keys of block j
                qa = qk_sb[i][0:64, :]
                qb = qk_sb[i][64:128, :]
                ka = qk_sb[4 + i][0:64, :]
                kb = qk_sb[4 + i][64:128, :]
                o = max(0, 128 * j - 512 * g)
                w = 512 - o
                q0 = 512 * g + o
                part = j >= 4 * g   # partial block: needs causal mask
                sab = psp.tile([128, 1024], f32, name=f"sab{i}_{g}_{j}",
                               tag="mm", bufs=3)
                nc.tensor.matmul(
                    sab[:, o:512], ka[:, j * 128:(j + 1) * 128],
                    qa[:, q0:q0 + w], start=True, stop=not part,
                    tile_position=(0, 0))
                nc.tensor.matmul(
                    sab[:, 512 + o:1024], kb[:, j * 128:(j + 1) * 128],
                    qb[:, q0:q0 + w], start=True, stop=not part,
                    tile_position=(64, 0))
                sab2 = sab[:].rearrange("p (h q) -> p h q", h=2)
                if part:
                    # add -30000 above the diagonal via identity matmul
                    # (both heads in one matmul: strided [128, 2, 128] out)
                    nc.tensor.matmul(
                        sab2[:, :, o:o + 128], ident_sb[:], maskb_sb[:],
                        start=False, stop=True, skip_group_check=True)
                if o > opair and not (g == 0 and j // 2 == 0):
                    # fill [opair, o) with -30000 so exp -> 0 (dead causal zone
                    # of the later block in a PV pair). start=False: the bank
                    # was already started by the S matmul (start=True clears
                    # the WHOLE 2KB bank); this accumulates onto pending-zero.
                    fw = o - opair
                    mv = neg30k_sb[:].rearrange("p (h q) -> p h q", h=2)[:, :, 0:fw]
                    nc.tensor.matmul(
                        sab2[:, :, opair:opair + fw], ident_sb[:],
                        mv, start=False, stop=True,
                        skip_group_check=True)
                jj = j // 2
                js = j % 2
                if js == 0:
                    p8 = ptp.tile([128, 2, 2, 512], f8, name=f"p8_{i}_{g}_{jj}",
                                  tag="p8")
                    p8s[(i, g, jj)] = p8
                p8 = p8s[(i, g, jj)]
                corner = (g == 0 and j == 0)
                e0 = 128 if (g == 0 and jj == 0) else opair
                # exp into fp8 P slot (both heads at once)
                sv = sab[:].rearrange("p (h q) -> p h q", h=2)[:, :, e0:512]
                pv_ = p8[:, js, :, e0:512]
                nc.scalar.activation(pv_, sv, Exp, scale=SCALE)
                if corner:
                    p16 = ptp16.tile([128, 2, 128], f16, name=f"p16_{i}",
                                     tag="p16")
                    sv0 = sab[:].rearrange("p (h q) -> p h q", h=2)[:, :, 0:128]
                    nc.scalar.activation(p16[:], sv0, Exp, scale=SCALE)
                    p16s[i] = p16

            def emit_attn_PV(i, g, jj, opair, first, last):
                # fp8 DoubleRow: two key blocks (2jj, 2jj+1) per instruction
                if first:
                    oa = psp.tile([65, 512], f32, name=f"oa{i}_{g}", tag="pva", bufs=1)
                    ob = psp.tile([65, 512], f32, name=f"ob{i}_{g}", tag="pvb", bufs=1)
                    accum[(i, g)] = (oa, ob)
                oa, ob = accum[(i, g)]
                p8 = p8s.pop((i, g, jj))
                va = v2_sb[jj][:, :, (2 * i) * 80:(2 * i) * 80 + 65]
                vb = v2_sb[jj][:, :, (2 * i + 1) * 80:(2 * i + 1) * 80 + 65]
                corner = (g == 0 and jj == 0)
                e0 = 128 if corner else opair
                nc.tensor.matmul(
                    oa[:, e0:512], va, p8[:, :, 0, e0:512],
                    start=first, stop=last, perf_mode=DR,
                    skip_group_check=True)
                nc.tensor.matmul(
                    ob[:, e0:512], vb, p8[:, :, 1, e0:512],
                    start=first, stop=last, perf_mode=DR,
                    skip_group_check=True)
                if corner:
                    # start=False: bank was started (and fully cleared) by the
                    # DR pair instruction above; accumulate onto pending-zero.
                    p16 = p16s.pop(i)
                    nc.tensor.matmul(
                        oa[:, 0:128], v16_sb[:, (2 * i) * 65:(2 * i) * 65 + 65],
                        p16[:, 0, :], start=False, stop=True,
                        skip_group_check=True)
                    nc.tensor.matmul(
                        ob[:, 0:128], v16_sb[:, (2 * i + 1) * 65:(2 * i + 1) * 65 + 65],
                        p16[:, 1, :], start=False, stop=True,
                        skip_group_check=True)

            def emit_attn_norm(i, g):
                # fast part: copy unnormalized O + denominators out of PSUM,
                # releasing the accumulators quickly
                oa, ob = accum.pop((i, g))
                sab_in = recp.tile([1, 1024], f32, name=f"sin{i}_{g}", tag="sin")
                g0 = 512 * g
                nc.vector.tensor_copy(oT_sb[i][0:64, g0:g0 + 512], oa[0:64, :])
                nc.vector.tensor_copy(oT_sb[i][64:128, g0:g0 + 512], ob[0:64, :])
                nc.vector.tensor_copy(sab_in[:, 0:512], oa[64:65, :])
                nc.vector.tensor_copy(sab_in[:, 512:1024], ob[64:65, :])
                sums[(i, g)] = sab_in

            def emit_attn_scale(i, g):
                # lazy part: 1/denom, broadcast, scale oT in place
                sab_in = sums.pop((i, g))
                rab = recp.tile([1, 1024], f32, name=f"rab{i}_{g}", tag="rab")
                rr = recp.tile([128, 512], f32, name=f"rr{i}_{g}", tag="rr")
                rbb = recp.tile([64, 512], f32, name=f"rbb{i}_{g}", tag="rbb")
                nc.vector.reciprocal_approx_fast(rab[:], sab_in[:])
                nc.gpsimd.partition_broadcast(rr[0:64, :], rab[:, 0:512], channels=64)
                nc.gpsimd.partition_broadcast(rbb[:], rab[:, 512:1024])
                nc.sync.dma_start(rr[64:128, :], rbb[:])
                g0 = 512 * g
                nc.vector.tensor_mul(
                    oT_sb[i][:, g0:g0 + 512],
                    oT_sb[i][:, g0:g0 + 512], rr[:])

            def emit_attn_gg(i, gg, vmap=None):
                # software-pipelined: each pair's PV is emitted after the NEXT
                # pair's S matmuls, so the exp latency hides behind PE work
                for g in (gg, gg + 1):
                    nj = 4 * g + 4
                    npair = nj // 2
                    pending = None
                    for jj in range(npair):
                        j0, j1 = 2 * jj, 2 * jj + 1
                        opair = max(0, 128 * j0 - 512 * g)
                        for j in (j0, j1):
                            if vmap and (g, j) in vmap:
                                emit_v(vmap[(g, j)])
                            emit_attn_S(i, g, j, opair)
                        if pending is not None:
                            emit_attn_PV(*pending)
                        pending = (i, g, jj, opair, jj == 0, jj == npair - 1)
                    emit_attn_PV(*pending)
                    emit_attn_norm(i, g)
                    emit_attn_scale(i, g)

            def emit_out(t):
                pt_ = psp.tile([128, 1024], f32, name=f"yp{t}", tag="mm", bufs=3)
                for k in range(4):
                    for h2 in range(2):
                        nc.tensor.matmul(
                            pt_[:, h2 * 512:(h2 + 1) * 512],
                            oT_sb[k][:, t * 128:(t + 1) * 128],
                            wo_sb[k][:, h2 * 512:(h2 + 1) * 512],
                            start=(k == 0), stop=(k == 3),
                        )
                yt = ysb.tile([128, D], f32, name=f"yt{t}", tag="yt")
                nc.vector.tensor_copy(yt[:], pt_[:])
                nc.sync.dma_start(y[t * 128:(t + 1) * 128, :], yt[:])

            # ---- interleaved emission ----
            emit_qk_row(0, 0); emit_qk_row(0, 1)
            emit_qk_row(4, 0); emit_qk_row(4, 1)
            emit_v(0); emit_v(1)
            emit_attn_gg(0, 0, vmap={(1, 4): 2, (1, 6): 3})
            emit_qk_row(1, 0); emit_qk_row(1, 1)
            emit_attn_gg(0, 2, vmap={(2, 8): 4, (2, 10): 5, (3, 12): 6, (3, 14): 7})
            emit_qk_row(5, 0); emit_qk_row(5, 1)
            emit_attn_gg(1, 0)
            emit_qk_row(2, 0); emit_qk_row(2, 1)
            emit_attn_gg(1, 2)
            emit_qk_row(6, 0); emit_qk_row(6, 1)
            emit_attn_gg(2, 0)
            emit_qk_row(3, 0); emit_qk_row(3, 1)
            emit_attn_gg(2, 2)
            emit_qk_row(7, 0); emit_qk_row(7, 1)
            emit_attn_gg(3, 0)
            for t in range(8):
                emit_out(t)
            emit_attn_gg(3, 2)
            for t in range(8, 16):
                emit_out(t)

    nc.compile()
    return nc


def _interleave_k(a):
    # [1024, N...] -> [512, 2, N...]: contraction k-pairs for DoubleRow
    # row (128*kk + p, g) <- original row 256*kk + 128*g + p
    n = a.shape[1:]
    return np.ascontiguousarray(
        a.reshape(4, 2, 128, *n).transpose(0, 2, 1, 3).reshape(512, 2, *n))


def _prep_inputs(x, w_qkv, b_qkv, w_out):
    import ml_dtypes
    f8 = ml_dtypes.float8_e4m3
    pidx = np.arange(128)
    maskb_np = np.where(pidx[None, :] >= pidx[:, None], 0.0, -30000.0).astype(np.float16)
    ident_np = np.eye(128, dtype=np.float16)
    in_maps = []
    for c in range(NCORES):
        b, hh = divmod(c, 2)
        h0 = hh * HPC * HD  # 0 or 512: offset into each of q/k/v col sections
        wq = w_qkv[:, h0:h0 + DH]
        wk = w_qkv[:, D + h0:D + h0 + DH]
        wv_ = w_qkv[:, 2 * D + h0:2 * D + h0 + DH]
        bq = b_qkv[h0:h0 + DH]
        bk = b_qkv[D + h0:D + h0 + DH]
        bv_ = b_qkv[2 * D + h0:2 * D + h0 + DH]
        xb = np.ascontiguousarray(x[b].T)  # [D, T]
        wqkc = np.concatenate([wq, wk], axis=1)
        in_maps.append({
            "x8": _interleave_k(xb).astype(f8),
            "wqk8": _interleave_k(wqkc * 32.0).astype(f8),
            "wv8": _interleave_k(wv_ * 32.0).astype(f8),
            "xc16": np.ascontiguousarray(xb[:, 0:128]).astype(np.float16),
            "wv": np.ascontiguousarray(wv_).astype(np.float16),
            "wo": np.ascontiguousarray(w_out[h0:h0 + DH, :]).astype(np.float16),
            "bqk": np.ascontiguousarray(
                np.concatenate([bq, bk]).reshape(8, 128).T).astype(np.float32),
            "bv": np.broadcast_to(bv_.astype(np.float32), (128, DH)).copy(),
            "maskb": maskb_np, "ident": ident_np,
        })
    return in_maps


def kernel(x, w_qkv, b_qkv, w_out, b_out, _trace=False, _trace_cores=None):
    from concourse.bass_utils import run_bass_kernel_spmd

    x = np.asarray(x, dtype=np.float32)
    w_qkv = np.asarray(w_qkv, dtype=np.float32)
    b_qkv = np.asarray(b_qkv, dtype=np.float32)
    w_out = np.asarray(w_out, dtype=np.float32)
    b_out = np.asarray(b_out, dtype=np.float32)

    if "nc" not in _cache:
        _cache["nc"] = _build_nc()
    nc = _cache["nc"]

    in_maps = _prep_inputs(x, w_qkv, b_qkv, w_out)
    res = run_bass_kernel_spmd(
        nc, in_maps, core_ids=list(range(NCORES)), trace=_trace,
        trace_cores=_trace_cores)
    _cache["last_result"] = res

    out = np.empty((B, T, D), dtype=np.float32)
    for b in range(B):
        out[b] = res.results[2 * b]["y"] + res.results[2 * b + 1]["y"]
    out += b_out[None, None, :].astype(np.float32)
    return out
